# revision 8
# baseline (speedup 1.0000x reference)
"""TRN2 kernel for nn_BClassifier: attention-MIL pooling on 8 NeuronCores.

Heavy stage (memory-bound): x [4, 20000, 512] attention pooling.
Sharding: 2 cores per bag, 10000 instance rows each. Each core computes
exp-softmax partials (sum_r e_r, sum_r e_r * x_r); host combines into the
pooled bag embedding M. The tiny 256-node hypergraph stage runs on host.
"""
import sys
import numpy as np

sys.path.insert(0, "/opt/trn_rl_repo")

from concourse import bass, bacc, mybir, tile, masks  # noqa: E402
from concourse import bass_utils  # noqa: E402

F32 = mybir.dt.float32

B, N, D, C = 4, 20000, 512, 4
NUM_NODE = 256
K = 8
EPS_GN = 1e-5
N_CORES = 8
R = (B * N) // N_CORES          # rows per core = 10000
ROW_TILE = 125                  # 80 tiles of exactly 125 rows
NT = R // ROW_TILE


def _build_mil_program():
    nc = bacc.Bacc(
        "TRN2",
        target_bir_lowering=False,
        debug=False,
        enable_asserts=False,
        num_devices=N_CORES,
    )
    x_d = nc.dram_tensor("x", [R, D], F32, kind="ExternalInput")
    aW1_d = nc.dram_tensor("aW1", [D, D], F32, kind="ExternalInput")
    ab1_d = nc.dram_tensor("ab1", [D], F32, kind="ExternalInput")
    aW2_d = nc.dram_tensor("aW2", [D, 1], F32, kind="ExternalInput")
    ab2_d = nc.dram_tensor("ab2", [1], F32, kind="ExternalInput")
    m_d = nc.dram_tensor("m_part", [1, D], F32, kind="ExternalOutput")
    s_d = nc.dram_tensor("s_part", [1, 1], F32, kind="ExternalOutput")

    with tile.TileContext(nc) as tc:
        with (
            tc.tile_pool(name="const", bufs=1) as cpool,
            tc.tile_pool(name="io", bufs=3) as iopool,
            tc.tile_pool(name="xt", bufs=3) as xtpool,
            tc.tile_pool(name="h1", bufs=3) as h1pool,
            tc.tile_pool(name="ev", bufs=3) as evpool,
            tc.tile_pool(name="ps", bufs=2, space=bass.MemorySpace.PSUM) as pspool,
            tc.tile_pool(name="acc", bufs=1, space=bass.MemorySpace.PSUM) as accpool,
        ):
            ident = cpool.tile([128, 128], F32)
            masks.make_identity(nc, ident[:])
            one_sb = cpool.tile([1, 1], F32)
            nc.gpsimd.memset(one_sb[:], 1.0)
            ones_col = cpool.tile([ROW_TILE, 1], F32)
            nc.gpsimd.memset(ones_col[:], 1.0)

            # aW1 [512,512] -> [128, 4*512]; K-chunk kc lives at [:, kc*512:(kc+1)*512]
            aW1_sb = cpool.tile([128, 4 * D], F32)
            for kc in range(4):
                nc.sync.dma_start(
                    aW1_sb[:, kc * D:(kc + 1) * D], aW1_d[kc * 128:(kc + 1) * 128, :]
                )
            # ab1 [512] -> [128, 4] (chunk kc at column kc)
            ab1_sb = cpool.tile([128, 4], F32)
            nc.sync.dma_start(ab1_sb[:, :], ab1_d.ap().rearrange("(a p) -> p a", p=128))
            # aW2 [512,1] -> [128, 4]
            aW2_sb = cpool.tile([128, 4], F32)
            nc.sync.dma_start(aW2_sb[:, :], aW2_d.ap().rearrange("(a p) o -> p (a o)", p=128))
            ab2_sb = cpool.tile([1, 1], F32)
            nc.sync.dma_start(ab2_sb[:, :], ab2_d.ap().rearrange("(a b) -> a b", a=1))

            m_acc = accpool.tile([1, D], F32)     # PSUM accumulator over all tiles
            s_acc = accpool.tile([1, 1], F32)

            for t in range(NT):
                x_sb = iopool.tile([ROW_TILE, D], F32, tag="x")
                nc.sync.dma_start(x_sb[:], x_d[t * ROW_TILE:(t + 1) * ROW_TILE, :])

                # transpose x tile -> xT [128 feat, 4*125] (chunk fc at cols fc*125)
                xT_ps = pspool.tile([128, 4 * ROW_TILE], F32, tag="xt_ps")
                for fc in range(4):
                    nc.tensor.transpose(
                        xT_ps[:, fc * ROW_TILE:(fc + 1) * ROW_TILE],
                        x_sb[:, fc * 128:(fc + 1) * 128],
                        ident[:ROW_TILE, :ROW_TILE],
                    )
                xT_sb = xtpool.tile([128, 4 * ROW_TILE], F32, tag="xt_sb")
                nc.any.tensor_copy(out=xT_sb[:], in_=xT_ps[:])

                # h1T = relu(aW1.T @ xT + ab1): [512 outfeat -> 4 chunks x 128, 125 rows]
                h1T_sb = h1pool.tile([128, 4 * ROW_TILE], F32, tag="h1t")
                for mo in range(4):
                    h1_ps = pspool.tile([128, ROW_TILE], F32, tag="h1_ps")
                    for kc in range(4):
                        nc.tensor.matmul(
                            h1_ps[:],
                            aW1_sb[:, kc * D + mo * 128: kc * D + (mo + 1) * 128],
                            xT_sb[:, kc * ROW_TILE:(kc + 1) * ROW_TILE],
                            start=(kc == 0),
                            stop=(kc == 3),
                        )
                    nc.scalar.activation(
                        h1T_sb[:, mo * ROW_TILE:(mo + 1) * ROW_TILE],
                        h1_ps[:],
                        mybir.ActivationFunctionType.Relu,
                        bias=ab1_sb[:, mo:mo + 1],
                    )

                # h2 = aW2.T @ h1T -> [1, 125]; e = exp(h2 + ab2)
                h2_ps = pspool.tile([1, ROW_TILE], F32, tag="small_ps")
                for mo in range(4):
                    nc.tensor.matmul(
                        h2_ps[:],
                        aW2_sb[:, mo:mo + 1],
                        h1T_sb[:, mo * ROW_TILE:(mo + 1) * ROW_TILE],
                        start=(mo == 0),
                        stop=(mo == 3),
                    )
                e_sb = evpool.tile([1, ROW_TILE], F32, tag="e")
                nc.scalar.activation(
                    e_sb[:], h2_ps[:], mybir.ActivationFunctionType.Exp,
                    bias=ab2_sb[0:1, 0:1],
                )

                # eT [125,1] = e.T via K=1 matmul with ones
                eT_ps = pspool.tile([ROW_TILE, 1], F32, tag="small_ps")
                nc.tensor.matmul(eT_ps[:], e_sb[:], one_sb[:], start=True, stop=True)
                eT_sb = evpool.tile([ROW_TILE, 1], F32, tag="eT_sb")
                nc.any.tensor_copy(out=eT_sb[:], in_=eT_ps[:])

                # accumulate m_part += e @ x_tile ; s_part += sum(e)
                nc.tensor.matmul(
                    m_acc[:], eT_sb[:], x_sb[:],
                    start=(t == 0), stop=(t == NT - 1), skip_group_check=True,
                )
                nc.tensor.matmul(
                    s_acc[:], eT_sb[:], ones_col[:],
                    start=(t == 0), stop=(t == NT - 1), skip_group_check=True,
                )

            m_out_sb = cpool.tile([1, D], F32)
            nc.any.tensor_copy(out=m_out_sb[:], in_=m_acc[:])
            nc.sync.dma_start(m_d[:, :], m_out_sb[:])
            s_out_sb = cpool.tile([1, 1], F32)
            nc.any.tensor_copy(out=s_out_sb[:], in_=s_acc[:])
            nc.sync.dma_start(s_d[:, :], s_out_sb[:])

    nc.compile()
    return nc


_NC_CACHE = {}


def _get_mil_program():
    if "mil" not in _NC_CACHE:
        _NC_CACHE["mil"] = _build_mil_program()
    return _NC_CACHE["mil"]


def _lrelu(x, s=0.01):
    return np.where(x >= 0, x, s * x)


def _graph_norm(x, w, b, ms):
    mean = x.mean(axis=0)
    out = x - mean * ms
    var = (out * out).mean(axis=0)
    return w * out / np.sqrt(var + EPS_GN) + b


def _hypergraph_conv_dense(x, he_attr, S, W, att, bias):
    """Dense form of PyG HypergraphConv (heads=1, attention) on the kNN
    hypergraph. S[i, j] = 1 iff j in nbr[i]; hyperedge i has the K nbrs of
    node i. Edge (dst=i, src=j) exists iff S[i,j]=1."""
    xw = x @ W                       # [256, F]
    hw = he_attr @ W                 # [256, F]
    d = att.shape[0] // 2
    p = xw @ att[:d]                 # [256] (src term)
    q = hw @ att[d:]                 # [256] (dst term)
    A = _lrelu(q[:, None] + p[None, :], 0.2)     # [i, j]
    mask = S > 0
    neg = np.where(mask, A, -np.inf)
    col_max = neg.max(axis=0)                    # per src node j
    col_max = np.where(np.isfinite(col_max), col_max, 0.0)
    E = np.where(mask, np.exp(A - col_max[None, :]), 0.0)
    colsum = E.sum(axis=0)
    colsum = np.where(colsum > 0, colsum, 1.0)
    alpha = E / colsum[None, :]
    deg = S.sum(axis=0)                          # node degree D (as src)
    Dinv = np.where(deg > 0, 1.0 / deg, 0.0)
    edge_feat = (alpha / K) @ xw                 # node -> hyperedge
    out = Dinv[:, None] * (alpha.T @ edge_feat)  # hyperedge -> node
    return out + bias


def _graph_stage(M, rehearsal, dW1, db1, dW2, db2,
                 h1W, h1att, h1b, h2W, h2att, h2b,
                 n1w, n1b, n1ms, n2w, n2b, n2ms,
                 f1W, f1b, f2W, f2b, gW1, gb1, gW2, gb2, cW, cb):
    x_concat = np.concatenate([M, rehearsal], axis=0)[:NUM_NODE]
    xf = _lrelu(_lrelu(x_concat @ dW1 + db1) @ dW2 + db2)
    norms = np.maximum(np.linalg.norm(xf, axis=1, keepdims=True), 1e-12)
    xn = xf / norms
    sim = xn @ xn.T
    # top-K per row via threshold on the 8th largest value
    kth = np.partition(sim, NUM_NODE - K, axis=1)[:, NUM_NODE - K]
    S = (sim >= kth[:, None]).astype(np.float32)
    edge_attr = (S @ xf) / K

    g1 = _hypergraph_conv_dense(xf, edge_attr, S, h1W, h1att, h1b)
    g1 = _lrelu(_graph_norm(g1, n1w, n1b, n1ms))
    out1 = _lrelu(g1 @ f1W + f1b)
    g2 = _hypergraph_conv_dense(g1, edge_attr, S, h2W, h2att, h2b)
    g2 = _lrelu(_graph_norm(g2, n2w, n2b, n2ms))
    out2 = _lrelu(g2 @ f2W + f2b)

    out = np.concatenate([xf, out1, out2], axis=1)   # [256, 1024]
    s = np.maximum(out.T @ gW1 + gb1, 0.0) @ gW2 + gb2
    s = 1.0 / (1.0 + np.exp(-s))
    s = s[:, 0] - np.mean(s)
    logits = (out * s[None, :]) @ cW + cb
    return logits


def kernel(**inputs):
    inp = {k: np.asarray(v) for k, v in inputs.items()}
    x = inp["x"].astype(np.float32)

    nc = _get_mil_program()
    in_maps = []
    for c in range(N_CORES):
        b, half = c // 2, c % 2
        in_maps.append({
            "x": np.ascontiguousarray(x[b].reshape(2, R, D)[half]),
            "aW1": np.ascontiguousarray(inp["aW1"].astype(np.float32)),
            "ab1": np.ascontiguousarray(inp["ab1"].astype(np.float32)),
            "aW2": np.ascontiguousarray(inp["aW2"].astype(np.float32)),
            "ab2": np.ascontiguousarray(inp["ab2"].astype(np.float32)),
        })
    res = bass_utils.run_bass_kernel_spmd(nc, in_maps, core_ids=list(range(N_CORES)))

    m = np.stack([res.results[c]["m_part"][0] for c in range(N_CORES)])  # [8, 512]
    s = np.array([res.results[c]["s_part"][0, 0] for c in range(N_CORES)])
    M = np.stack([(m[2 * b] + m[2 * b + 1]) / (s[2 * b] + s[2 * b + 1])
                  for b in range(B)]).astype(np.float32)                 # [4, 512]

    logits_mlp = (M @ inp["bagW"] + inp["bagB"]).astype(np.float32)

    gkeys = ["rehearsal", "dW1", "db1", "dW2", "db2",
             "h1W", "h1att", "h1b", "h2W", "h2att", "h2b",
             "n1w", "n1b", "n1ms", "n2w", "n2b", "n2ms",
             "f1W", "f1b", "f2W", "f2b", "gW1", "gb1", "gW2", "gb2", "cW", "cb"]
    logits = _graph_stage(M, *[inp[k].astype(np.float32) for k in gkeys])
    logits_graph = logits[:B].astype(np.float32)
    return logits_mlp, logits_graph


# revision 17
# speedup vs baseline: 2.8579x; 2.8579x over previous
"""TRN2 kernel for nn_BClassifier: attention-MIL pooling on 8 NeuronCores.

Heavy stage (memory-bound): x [4, 20000, 512] attention pooling.
Sharding: 2 cores per bag, 10000 instance rows each. Each core computes
exp-softmax partials (sum_r e_r, sum_r e_r * x_r); host combines into the
pooled bag embedding M. The tiny 256-node hypergraph stage runs on host.

V2: bf16 matmuls (f32 runs 2-pass LOW_HIGH on PE), 500-row blocks so the
h1 matmuls stream N=500, s-partials via Exp accum_out instead of matmuls.
"""
import sys
import numpy as np

sys.path.insert(0, "/opt/trn_rl_repo")

from concourse import bass, bacc, mybir, tile, masks  # noqa: E402
from concourse import bass_utils  # noqa: E402

F32 = mybir.dt.float32
BF16 = mybir.dt.bfloat16
AF = mybir.ActivationFunctionType

B, N, D, C = 4, 20000, 512, 4
NUM_NODE = 256
K = 8
EPS_GN = 1e-5
N_CORES = 8
R = (B * N) // N_CORES          # rows per core = 10000
SUB = 125                       # rows per partition tile
NSUB = 4                        # sub-tiles per block
BLK = SUB * NSUB                # 500 rows per block
NBLK = R // BLK                 # 20 blocks


def _build_mil_program():
    nc = bacc.Bacc(
        "TRN2",
        target_bir_lowering=False,
        debug=False,
        enable_asserts=False,
        num_devices=N_CORES,
    )
    x_d = nc.dram_tensor("x", [R, D], F32, kind="ExternalInput")
    aW1_d = nc.dram_tensor("aW1", [D, D], F32, kind="ExternalInput")
    ab1_d = nc.dram_tensor("ab1", [D], F32, kind="ExternalInput")
    aW2_d = nc.dram_tensor("aW2", [D, 1], F32, kind="ExternalInput")
    ab2_d = nc.dram_tensor("ab2", [1], F32, kind="ExternalInput")
    m_d = nc.dram_tensor("m_part", [1, D], F32, kind="ExternalOutput")
    s_d = nc.dram_tensor("s_part", [1, NBLK], F32, kind="ExternalOutput")

    with tile.TileContext(nc) as tc:
        with (
            tc.tile_pool(name="const", bufs=1) as cpool,
            tc.tile_pool(name="io", bufs=2 * NSUB) as iopool,
            tc.tile_pool(name="xt", bufs=2) as xtpool,
            tc.tile_pool(name="h1", bufs=2) as h1pool,
            tc.tile_pool(name="ev", bufs=2) as evpool,
            tc.tile_pool(name="pst", bufs=2, space=bass.MemorySpace.PSUM) as pstpool,
            tc.tile_pool(name="ps", bufs=2, space=bass.MemorySpace.PSUM) as pspool,
            tc.tile_pool(name="pssm", bufs=1, space=bass.MemorySpace.PSUM) as pssmpool,
            tc.tile_pool(name="acc", bufs=1, space=bass.MemorySpace.PSUM) as accpool,
        ):
            ident = cpool.tile([128, 128], BF16)
            masks.make_identity(nc, ident[:])
            one_sb = cpool.tile([1, 1], F32)
            nc.gpsimd.memset(one_sb[:], 1.0)

            # weights: DMA f32 then cast to bf16 once
            aW1_f = cpool.tile([128, 4 * D], F32)
            for kc in range(4):
                nc.sync.dma_start(
                    aW1_f[:, kc * D:(kc + 1) * D], aW1_d[kc * 128:(kc + 1) * 128, :]
                )
            aW1_sb = cpool.tile([128, 4 * D], BF16)
            nc.vector.tensor_copy(aW1_sb[:], aW1_f[:])
            ab1_sb = cpool.tile([128, 4], F32)
            nc.sync.dma_start(ab1_sb[:, :], ab1_d.ap().rearrange("(a p) -> p a", p=128))
            aW2_f = cpool.tile([128, 4], F32)
            nc.sync.dma_start(aW2_f[:, :], aW2_d.ap().rearrange("(a p) o -> p (a o)", p=128))
            aW2_sb = cpool.tile([128, 4], BF16)
            nc.vector.tensor_copy(aW2_sb[:], aW2_f[:])
            ab2_sb = cpool.tile([1, 1], F32)
            nc.sync.dma_start(ab2_sb[:, :], ab2_d.ap().rearrange("(a b) -> a b", a=1))

            s_sb = cpool.tile([1, NBLK], F32)
            m_acc = accpool.tile([1, D], F32)

            for blk in range(NBLK):
                r0 = blk * BLK
                x_bf = []
                for t in range(NSUB):
                    x_f = iopool.tile([SUB, D], F32, tag="x")
                    nc.sync.dma_start(
                        x_f[:], x_d[r0 + t * SUB:r0 + (t + 1) * SUB, :]
                    )
                    xb = iopool.tile([SUB, D], BF16, tag="xb")
                    # split casts between DVE and ACT to balance engines
                    if t % 2 == 0:
                        nc.vector.tensor_copy(xb[:], x_f[:])
                    else:
                        nc.scalar.copy(xb[:], x_f[:])
                    x_bf.append(xb)

                # transpose block -> xT bf16, PADDED layout: sub-block (kc, t) at
                # column (kc*4+t)*128 (256B-aligned PSUM writes); cols 125..127 pad
                xT_ps = pstpool.tile([128, 16 * 128], BF16, tag="xt_ps")
                for kc in range(4):
                    for t in range(NSUB):
                        c0 = (kc * NSUB + t) * 128
                        nc.tensor.transpose(
                            xT_ps[:, c0:c0 + SUB],
                            x_bf[t][:, kc * 128:(kc + 1) * 128],
                            ident[:SUB, :SUB],
                        )
                xT_sb = xtpool.tile([128, 16 * 128], BF16, tag="xt_sb")
                nc.any.tensor_copy(out=xT_sb[:], in_=xT_ps[:])
                xT_3d = xT_sb[:].rearrange("p (a b) -> p a b", a=16)

                # h1T = relu(aW1.T @ xT + ab1), same padded layout, bf16
                h1T_sb = h1pool.tile([128, 16 * 128], BF16, tag="h1t")
                h1T_3d = h1T_sb[:].rearrange("p (a b) -> p a b", a=16)
                for mo in range(4):
                    h1_ps = pspool.tile([128, NSUB * 128], F32, tag="h1_ps")
                    h1_ps3 = h1_ps[:].rearrange("p (a b) -> p a b", a=NSUB)
                    for kc in range(4):
                        nc.tensor.matmul(
                            h1_ps3[:, :, 0:SUB],
                            aW1_sb[:, kc * D + mo * 128: kc * D + (mo + 1) * 128],
                            xT_3d[:, kc * NSUB:(kc + 1) * NSUB, 0:SUB],
                            start=(kc == 0),
                            stop=(kc == 3),
                        )
                    nc.scalar.activation(
                        h1T_3d[:, mo * NSUB:(mo + 1) * NSUB, 0:SUB],
                        h1_ps3[:, :, 0:SUB],
                        AF.Relu,
                        bias=ab1_sb[:, mo:mo + 1],
                    )

                # h2 = aW2.T @ h1T -> [1, 500]; e = exp(h2 + ab2), accum -> s
                h2_ps = pssmpool.tile([1, NSUB * 128], F32, tag="small_ps")
                h2_ps3 = h2_ps[:].rearrange("p (a b) -> p a b", a=NSUB)
                for mo in range(4):
                    nc.tensor.matmul(
                        h2_ps3[:, :, 0:SUB],
                        aW2_sb[:, mo:mo + 1],
                        h1T_3d[:, mo * NSUB:(mo + 1) * NSUB, 0:SUB],
                        start=(mo == 0),
                        stop=(mo == 3),
                    )
                e_sb = evpool.tile([1, NSUB * 128], F32, tag="e")
                e_3d = e_sb[:].rearrange("p (a b) -> p a b", a=NSUB)
                nc.scalar.activation(
                    e_3d[:, :, 0:SUB], h2_ps3[:, :, 0:SUB], AF.Exp,
                    bias=ab2_sb[0:1, 0:1],
                    accum_out=s_sb[0:1, blk:blk + 1],
                )

                # eT [125, 4] via K=1 matmuls (f32, tiny), cast to bf16
                eT_ps = pssmpool.tile([SUB, NSUB], F32, tag="small_ps")
                for t in range(NSUB):
                    nc.tensor.matmul(
                        eT_ps[:, t:t + 1],
                        e_sb[:, t * 128:t * 128 + SUB],
                        one_sb[:],
                        start=True, stop=True,
                    )
                eT_bf = evpool.tile([SUB, NSUB], BF16, tag="eT_bf")
                nc.any.tensor_copy(out=eT_bf[:], in_=eT_ps[:])

                # m_part += e_t @ x_t for each sub-tile
                for t in range(NSUB):
                    nc.tensor.matmul(
                        m_acc[:], eT_bf[:, t:t + 1], x_bf[t][:],
                        start=(blk == 0 and t == 0),
                        stop=(blk == NBLK - 1 and t == NSUB - 1),
                        skip_group_check=True,
                    )

            m_out_sb = cpool.tile([1, D], F32)
            nc.any.tensor_copy(out=m_out_sb[:], in_=m_acc[:])
            nc.sync.dma_start(m_d[:, :], m_out_sb[:])
            nc.sync.dma_start(s_d[:, :], s_sb[:])

    nc.compile()
    return nc


_NC_CACHE = {}


def _get_mil_program():
    if "mil" not in _NC_CACHE:
        _NC_CACHE["mil"] = _build_mil_program()
    return _NC_CACHE["mil"]


def _lrelu(x, s=0.01):
    return np.where(x >= 0, x, s * x)


def _graph_norm(x, w, b, ms):
    mean = x.mean(axis=0)
    out = x - mean * ms
    var = (out * out).mean(axis=0)
    return w * out / np.sqrt(var + EPS_GN) + b


def _hypergraph_conv_dense(x, he_attr, S, W, att, bias):
    """Dense form of PyG HypergraphConv (heads=1, attention) on the kNN
    hypergraph. S[i, j] = 1 iff j in nbr[i]; hyperedge i has the K nbrs of
    node i. Edge (dst=i, src=j) exists iff S[i,j]=1."""
    xw = x @ W                       # [256, F]
    hw = he_attr @ W                 # [256, F]
    d = att.shape[0] // 2
    p = xw @ att[:d]                 # [256] (src term)
    q = hw @ att[d:]                 # [256] (dst term)
    A = _lrelu(q[:, None] + p[None, :], 0.2)     # [i, j]
    mask = S > 0
    neg = np.where(mask, A, -np.inf)
    col_max = neg.max(axis=0)                    # per src node j
    col_max = np.where(np.isfinite(col_max), col_max, 0.0)
    E = np.where(mask, np.exp(A - col_max[None, :]), 0.0)
    colsum = E.sum(axis=0)
    colsum = np.where(colsum > 0, colsum, 1.0)
    alpha = E / colsum[None, :]
    deg = S.sum(axis=0)                          # node degree D (as src)
    Dinv = np.where(deg > 0, 1.0 / deg, 0.0)
    edge_feat = (alpha / K) @ xw                 # node -> hyperedge
    out = Dinv[:, None] * (alpha.T @ edge_feat)  # hyperedge -> node
    return out + bias


def _graph_stage(M, rehearsal, dW1, db1, dW2, db2,
                 h1W, h1att, h1b, h2W, h2att, h2b,
                 n1w, n1b, n1ms, n2w, n2b, n2ms,
                 f1W, f1b, f2W, f2b, gW1, gb1, gW2, gb2, cW, cb):
    x_concat = np.concatenate([M, rehearsal], axis=0)[:NUM_NODE]
    xf = _lrelu(_lrelu(x_concat @ dW1 + db1) @ dW2 + db2)
    norms = np.maximum(np.linalg.norm(xf, axis=1, keepdims=True), 1e-12)
    xn = xf / norms
    sim = xn @ xn.T
    # top-K per row via threshold on the 8th largest value
    kth = np.partition(sim, NUM_NODE - K, axis=1)[:, NUM_NODE - K]
    S = (sim >= kth[:, None]).astype(np.float32)
    edge_attr = (S @ xf) / K

    g1 = _hypergraph_conv_dense(xf, edge_attr, S, h1W, h1att, h1b)
    g1 = _lrelu(_graph_norm(g1, n1w, n1b, n1ms))
    out1 = _lrelu(g1 @ f1W + f1b)
    g2 = _hypergraph_conv_dense(g1, edge_attr, S, h2W, h2att, h2b)
    g2 = _lrelu(_graph_norm(g2, n2w, n2b, n2ms))
    out2 = _lrelu(g2 @ f2W + f2b)

    out = np.concatenate([xf, out1, out2], axis=1)   # [256, 1024]
    s = np.maximum(out.T @ gW1 + gb1, 0.0) @ gW2 + gb2
    s = 1.0 / (1.0 + np.exp(-s))
    s = s[:, 0] - np.mean(s)
    logits = (out * s[None, :]) @ cW + cb
    return logits


def kernel(**inputs):
    inp = {k: np.asarray(v) for k, v in inputs.items()}
    x = inp["x"].astype(np.float32)

    nc = _get_mil_program()
    in_maps = []
    for c in range(N_CORES):
        b, half = c // 2, c % 2
        in_maps.append({
            "x": np.ascontiguousarray(x[b].reshape(2, R, D)[half]),
            "aW1": np.ascontiguousarray(inp["aW1"].astype(np.float32)),
            "ab1": np.ascontiguousarray(inp["ab1"].astype(np.float32)),
            "aW2": np.ascontiguousarray(inp["aW2"].astype(np.float32)),
            "ab2": np.ascontiguousarray(inp["ab2"].astype(np.float32)),
        })
    res = bass_utils.run_bass_kernel_spmd(nc, in_maps, core_ids=list(range(N_CORES)))

    m = np.stack([res.results[c]["m_part"][0] for c in range(N_CORES)])  # [8, 512]
    s = np.array([res.results[c]["s_part"].sum() for c in range(N_CORES)])
    M = np.stack([(m[2 * b] + m[2 * b + 1]) / (s[2 * b] + s[2 * b + 1])
                  for b in range(B)]).astype(np.float32)                 # [4, 512]

    logits_mlp = (M @ inp["bagW"] + inp["bagB"]).astype(np.float32)

    gkeys = ["rehearsal", "dW1", "db1", "dW2", "db2",
             "h1W", "h1att", "h1b", "h2W", "h2att", "h2b",
             "n1w", "n1b", "n1ms", "n2w", "n2b", "n2ms",
             "f1W", "f1b", "f2W", "f2b", "gW1", "gb1", "gW2", "gb2", "cW", "cb"]
    logits = _graph_stage(M, *[inp[k].astype(np.float32) for k in gkeys])
    logits_graph = logits[:B].astype(np.float32)
    return logits_mlp, logits_graph
